# revision 15
# baseline (speedup 1.0000x reference)
"""DeepseekV2 MoE layer on 8 Trainium2 NeuronCores (Bass/Tile, SPMD).

Strategy (expert-parallel, all-bf16 matmuls, fp32 PSUM accumulate):
 - Host computes the MoE gate routing in numpy (bitwise-matches the jax
   reference: top-k margins are ~1e-4, far above ulp noise).
 - 16 experts -> 8 cores x 3 slots.  Slot capacities (C0>=C1>=C2) are
   chosen by a small DP so that expert token loads (which are heavily
   imbalanced) pack into the 24 slots with minimal total capacity;
   overfull experts are SPLIT across slots on different cores.  SPMD
   cores run an identical instruction stream, so per-core cost is
   C0+C1+C2 regardless of data - minimizing that sum minimizes time.
 - Per slot: GEMM1 (wgu tiles stationary, gathered x^T moving) -> SiLU
   on ScalarE -> *up *cw on VectorE -> GEMM2 *transposed* (w_down^T
   tiles stationary, activations moving) producing y^T [D, cap] so
   capacities need no 128-row padding.  The routed gate weight (incl.
   the 2.5 scale) is applied via a host-replicated [128, cap] row.
 - Shared expert: TP-sharded over its intermediate dim (352/core,
   padded to 3x128), same transposed pipeline, partials summed on host.
 - bf16 operands halve HBM traffic vs fp32 (the baseline bottleneck);
   matmul rate is 1 cycle/row either way.  Measured rel err ~4e-3.
 - Host scatter-adds per-piece outputs and sums shared partials.
"""

import itertools
import numpy as np
import ml_dtypes
from contextlib import ExitStack

import concourse.bacc as bacc
import concourse.tile as tile
import concourse.mybir as mybir
from concourse.bass_utils import run_bass_kernel_spmd

# problem dims (fixed by the graded problem)
T, D, I, E = 1024, 2048, 1408, 16
SI = 2 * I               # shared expert intermediate (2816)
TOP_K, N_GROUP, TOPK_GROUP = 6, 4, 2
ROUTED_SCALE = 2.5
NCORES = 8
KT = D // 128            # 16 contraction tiles (gemm1)
IT = I // 128            # 11 intermediate tiles (routed)
DT = D // 128            # 16 output d-tiles (gemm2)
SSLICE = SI // NCORES    # 352 shared-intermediate rows per core
SIP = 384                # padded to 3x128
SIT = SIP // 128         # 3

f32 = mybir.dt.float32
bf16 = mybir.dt.bfloat16
BF = ml_dtypes.bfloat16
ACT_SILU = mybir.ActivationFunctionType.Silu


# ---------------------------------------------------------------- routing
def _route(x, gate_w, bias):
    """Replicates the jax reference gate in numpy f32 (decision margins are
    >=1e-4 so ulp-level differences cannot flip the top-k).

    Returns topk_idx [T,6] int, weights [T,6] f32 (renormalized, unscaled).
    """
    logits = (x @ gate_w.T).astype(np.float32)
    scores = (1.0 / (1.0 + np.exp(-logits))).astype(np.float32)
    s_choice = scores + bias.astype(np.float32)
    grp = s_choice.reshape(T, N_GROUP, E // N_GROUP)
    group_scores = np.sort(grp, axis=2)[:, :, -2:].sum(2, dtype=np.float32)
    grp_idx = np.argsort(-group_scores, axis=1, kind="stable")[:, :TOPK_GROUP]
    gmask = np.zeros((T, N_GROUP), dtype=bool)
    gmask[np.arange(T)[:, None], grp_idx] = True
    emask = np.repeat(gmask, E // N_GROUP, axis=1)
    masked = np.where(emask, s_choice, -np.inf)
    topk_idx = np.argsort(-masked, axis=1, kind="stable")[:, :TOP_K]
    w = np.take_along_axis(scores, topk_idx, axis=1)
    w = (w / w.sum(axis=1, keepdims=True)).astype(np.float32)
    return topk_idx, w


# --------------------------------------------------------- slot assignment
def _feasible(caps, counts, max_pieces=3):
    """Can each expert be covered by <=max_pieces slots (8 per capacity
    class) with total capacity >= its token count?  Returns per-expert
    class-use tuples or None."""
    nclass = len(caps)
    opts_per_expert = []
    for cnt in counts:
        opts = []
        for a in itertools.product(range(max_pieces + 1), repeat=nclass):
            if sum(a) == 0 or sum(a) > max_pieces:
                continue
            tot = sum(ai * c for ai, c in zip(a, caps))
            if tot >= cnt:
                dom = any(a[j] > 0 and tot - caps[j] >= cnt for j in range(nclass))
                if not dom:
                    opts.append(a)
        if not opts:
            return None
        opts_per_expert.append(opts)
    states = {(0,) * nclass: []}
    for opts in opts_per_expert:
        new = {}
        for st, hist in states.items():
            for a in opts:
                nst = tuple(s + ai for s, ai in zip(st, a))
                if all(n <= NCORES for n in nst) and nst not in new:
                    new[nst] = hist + [a]
        if not new:
            return None
        states = new
    return next(iter(states.values()))


def _solve_slots(counts):
    """Pick 3 slot capacities (multiples of 8, each <=512) minimizing
    C0+C1+C2 s.t. the expert loads pack into 8 slots per class.
    Returns (caps, per-expert class-use tuples)."""
    total = int(counts.sum())
    lo = -(-total // NCORES)
    lo = -(-lo // 8) * 8
    for s in range(lo, 1537, 8):
        for C0 in range(-(-s // 3) // 8 * 8, min(512, s - 16) + 1, 8):
            for C1 in range(8, min(C0, s - C0 - 8) + 1, 8):
                C2 = s - C0 - C1
                if C2 < 8 or C2 > C1:
                    continue
                r = _feasible((C0, C1, C2), counts)
                if r is not None:
                    return [C0, C1, C2], r
    raise RuntimeError("no feasible slot packing")


# ------------------------------------------------------------ host packing
def _pack_wgu(w2, it_cnt):
    """w2: [2*ic, D] rows (gate block then up block, ic=128*it_cnt rows)
    -> [it_cnt, 128, 2, KT, 128] bf16: tile (t, j) is wgu^T laid out so
    [p, c] = w2[j_block + t*128 + c, k*128 + p] (lhsT k-slices)."""
    ic = 128 * it_cnt
    g = w2[:ic].reshape(it_cnt, 128, D)
    u = w2[ic:].reshape(it_cnt, 128, D)
    arr = np.stack([g, u], axis=1)               # (t, j, c, D)
    arr = arr.reshape(it_cnt, 2, 128, KT, 128)   # (t, j, c, k, p)
    return np.ascontiguousarray(arr.transpose(0, 4, 1, 3, 2), dtype=BF)


def _pack_wdT(wd, it_cnt):
    """wd: [D, 128*it_cnt] (w_down, cols may be zero padded)
    -> [DT, 128, it_cnt, 128] bf16: tile (dt, k) is lhsT with
    [i, d] = wd[dt*128 + d, k*128 + i]."""
    a = wd.reshape(DT, 128, it_cnt, 128)         # (dt, d, k, i)
    return np.ascontiguousarray(a.transpose(0, 3, 2, 1), dtype=BF)


def _pack_xT(xs, cap):
    """xs: [n, D] token rows -> [128, KT, cap] bf16 (x^T k-tiles, padded)."""
    out = np.zeros((128, KT, cap), dtype=BF)
    n = xs.shape[0]
    if n:
        out[:, :, :n] = xs.T.reshape(KT, 128, n).transpose(1, 0, 2).astype(BF)
    return out


# ------------------------------------------------------------ device build
def _build(caps):
    nc = bacc.Bacc("TRN2", target_bir_lowering=False, debug=False,
                   num_devices=NCORES)
    S = len(caps)
    xg_d = [nc.dram_tensor(f"xg{s}", [128, KT, caps[s]], bf16, kind="ExternalInput") for s in range(S)]
    cw_d = [nc.dram_tensor(f"cw{s}", [128, caps[s]], f32, kind="ExternalInput") for s in range(S)]
    wgu_d = [nc.dram_tensor(f"wgu{s}", [IT, 128, 2, KT, 128], bf16, kind="ExternalInput") for s in range(S)]
    wd_d = [nc.dram_tensor(f"wd{s}", [DT, 128, IT, 128], bf16, kind="ExternalInput") for s in range(S)]
    yr_d = [nc.dram_tensor(f"yr{s}", [DT, 128, caps[s]], bf16, kind="ExternalOutput") for s in range(S)]
    xt_d = nc.dram_tensor("xt", [128, KT, T], bf16, kind="ExternalInput")
    wsgu_d = nc.dram_tensor("wsgu", [SIT, 128, 2, KT, 128], bf16, kind="ExternalInput")
    wsd_d = nc.dram_tensor("wsd", [DT, 128, SIT, 128], bf16, kind="ExternalInput")
    ys_d = nc.dram_tensor("ys", [DT, 128, T], bf16, kind="ExternalOutput")

    with tile.TileContext(nc) as tc, ExitStack() as ctx:
        sb = ctx.enter_context(tc.tile_pool(name="sb", bufs=1))
        ps = ctx.enter_context(tc.tile_pool(name="ps", bufs=1, space="PSUM"))

        # Phase order: shared expert FIRST (its weight stream is only
        # ~80GB/s vs the routed phases' 220-390GB/s), so its window
        # prefetches the routed weights; then routed slots smallest-
        # capacity first (s2 has the hungriest weight stream and gets
        # the shared window's prefetch), each phase's gemm2 window
        # loading the next phase's x and gemm2 weights (all HWDGE).
        q = KT // 4
        xgs = [sb.tile([128, KT, caps[s]], bf16, tag="xg", bufs=3, name=f"xga{s}")
               for s in range(S)]
        cws = [sb.tile([128, caps[s]], f32, tag="cw", bufs=3, name=f"cwa{s}")
               for s in range(S)]
        xt = sb.tile([128, KT, T], bf16, tag="xt", bufs=1, name="xt")

        def load_xg(s):
            for i in range(4):
                nc.scalar.dma_start(xgs[s][:, i * q:(i + 1) * q, :],
                                    xg_d[s].ap()[:, i * q:(i + 1) * q, :])
            nc.scalar.dma_start(cws[s][:], cw_d[s].ap()[:])

        # xt pieces ordered to match the first gemm1 chunk's consumption
        # (all k of columns 0-511 first)
        h = KT // 2
        for c0, ks in ((0, (0, h)), (0, (h, KT)), (512, (0, h)), (512, (h, KT))):
            nc.scalar.dma_start(xt[:, ks[0]:ks[1], c0:c0 + 512],
                                xt_d.ap()[:, ks[0]:ks[1], c0:c0 + 512])
        # gemm2 weight tiles: shared's on their own small tag; the routed
        # phases' share one ring whose rotation paces each phase's load
        # off the previous phase's gemm2 progress.
        wsd_tiles = [sb.tile([128, SIT, 128], bf16, tag="wsd", bufs=16,
                             name="wsd") for _ in range(DT)]
        wd_tiles = {}
        for s in (2, 1, 0):
            wd_tiles[s] = [sb.tile([128, IT, 128], bf16, tag="wd2", bufs=16,
                                   name="wdr") for _ in range(DT)]

        def load_wsd():
            for dt in range(DT):
                nc.scalar.dma_start(wsd_tiles[dt][:], wsd_d.ap()[dt])

        def load_wd(s):
            for dt in range(DT):
                nc.scalar.dma_start(wd_tiles[s][dt][:], wd_d[s].ap()[dt])

        def ffn(xg, cwB, wgu_dram, wds, out_dram, C, it_cnt, chunks,
                prefetch=None, out_dt=f32):
            # GEMM1 + silu*up*cw -> at (activations^T, [i, tokens], bf16)
            at = sb.tile([128, it_cnt, C], bf16, tag="at", bufs=2, name="at")
            for t in range(it_cnt):
                wgu = sb.tile([128, 2, KT, 128], bf16, tag="wgu", bufs=6, name="wgu")
                nc.sync.dma_start(wgu[:, 0], wgu_dram.ap()[t][:, 0])
                nc.sync.dma_start(wgu[:, 1], wgu_dram.ap()[t][:, 1])
                for off, n in chunks:
                    psg = ps.tile([128, n], f32, tag="psg", bufs=3, name="psg")
                    psu = ps.tile([128, n], f32, tag="psu", bufs=3, name="psu")
                    for j, p in ((0, psg), (1, psu)):
                        for k in range(KT):
                            nc.tensor.matmul(p[:], wgu[:, j, k, :],
                                             xg[:, k, off:off + n],
                                             start=(k == 0), stop=(k == KT - 1))
                    tmp = sb.tile([128, n], f32, tag="tmp", bufs=2, name="tmp")
                    nc.scalar.activation(tmp[:], psg[:], ACT_SILU)
                    if cwB is not None:
                        tmp2 = sb.tile([128, n], f32, tag="tmp2", bufs=2, name="tmp2")
                        nc.vector.tensor_mul(tmp2[:], tmp[:], psu[:])
                        nc.vector.tensor_mul(at[:, t, off:off + n], tmp2[:],
                                             cwB[:, off:off + n])
                    else:
                        nc.vector.tensor_mul(at[:, t, off:off + n], tmp[:], psu[:])
            if prefetch is not None:
                prefetch()
            # GEMM2 (transposed): y^T[d-tile] = sum_k wdT[dt,k].T @ at[k]
            for dt in range(DT):
                wd = wds[dt]
                for off, n in chunks:
                    psy = ps.tile([128, n], f32, tag="psy", bufs=2, name="psy")
                    for k in range(it_cnt):
                        nc.tensor.matmul(psy[:], wd[:, k, :], at[:, k, off:off + n],
                                         start=(k == 0), stop=(k == it_cnt - 1))
                    ysb = sb.tile([128, n], out_dt, tag="ysb", bufs=3, name="ysb")
                    nc.vector.tensor_copy(ysb[:], psy[:])
                    nc.scalar.dma_start(out_dram.ap()[dt][:, off:off + n], ysb[:])

        ffn(xt, None, wsgu_d, wsd_tiles, ys_d, T, SIT, [(0, 512), (512, 512)],
            prefetch=lambda: (load_wsd(), load_xg(2), load_wd(2)),
            out_dt=bf16)
        ffn(xgs[2], cws[2], wgu_d[2], wd_tiles[2], yr_d[2], caps[2], IT,
            [(0, caps[2])], prefetch=lambda: (load_xg(1), load_wd(1)),
            out_dt=bf16)
        ffn(xgs[1], cws[1], wgu_d[1], wd_tiles[1], yr_d[1], caps[1], IT,
            [(0, caps[1])], prefetch=lambda: (load_xg(0), load_wd(0)),
            out_dt=bf16)
        ffn(xgs[0], cws[0], wgu_d[0], wd_tiles[0], yr_d[0], caps[0], IT,
            [(0, caps[0])], out_dt=bf16)

    nc.compile()
    return nc


# ----------------------------------------------------------------- kernel
def kernel(x, gate_w, bias, w_gate_up, w_down, shared_w_gate_up,
           shared_w_down, _trace=False):
    x = np.ascontiguousarray(x, dtype=np.float32)
    topk_idx, w = _route(x, gate_w, bias)
    cw_full = w.astype(np.float32) * np.float32(ROUTED_SCALE)

    # expert -> token list + weight list
    toks, wts, counts = [], [], np.zeros(E, dtype=np.int64)
    for e in range(E):
        tsel, ksel = np.where(topk_idx == e)
        toks.append(tsel)
        wts.append(cw_full[tsel, ksel])
        counts[e] = len(tsel)

    caps, uses = _solve_slots(counts)
    S = len(caps)
    # build per-class piece lists: (expert, token_idx_array, weight_array)
    class_pieces = [[] for _ in range(S)]
    for e in range(E):
        pos = 0
        # fill this expert's pieces largest-class first
        for s in range(S):
            for _ in range(uses[e][s]):
                n = min(caps[s], counts[e] - pos)
                class_pieces[s].append((e, toks[e][pos:pos + n], wts[e][pos:pos + n]))
                pos += n
    for s in range(S):
        while len(class_pieces[s]) < NCORES:   # dummy empty pieces
            class_pieces[s].append((0, toks[0][:0], wts[0][:0]))

    # pre-pack each expert's weights once (pieces share the arrays)
    wgu_pack = {}
    wd_pack = {}
    for s in range(S):
        for e, _, _ in class_pieces[s]:
            if e not in wgu_pack:
                wgu_pack[e] = _pack_wgu(w_gate_up[e], IT)
                wd_pack[e] = _pack_wdT(np.ascontiguousarray(w_down[e]), IT)
    xt_arr = _pack_xT(x, T)

    in_maps = []
    for c in range(NCORES):
        m = {}
        for s in range(S):
            e, ptoks, pwts = class_pieces[s][c]
            m[f"xg{s}"] = _pack_xT(x[ptoks], caps[s])
            cwb = np.zeros((128, caps[s]), dtype=np.float32)
            cwb[:, :len(pwts)] = pwts[None, :]
            m[f"cw{s}"] = cwb
            m[f"wgu{s}"] = wgu_pack[e]
            m[f"wd{s}"] = wd_pack[e]
        # shared expert slice (rows [352c, 352c+352), zero-padded to 384)
        gsl = np.zeros((2 * SIP, D), dtype=np.float32)
        gsl[:SSLICE] = shared_w_gate_up[SSLICE * c: SSLICE * (c + 1)]
        gsl[SIP:SIP + SSLICE] = shared_w_gate_up[SI + SSLICE * c: SI + SSLICE * (c + 1)]
        m["wsgu"] = _pack_wgu(gsl, SIT)
        sds = np.zeros((D, SIP), dtype=np.float32)
        sds[:, :SSLICE] = shared_w_down[:, SSLICE * c: SSLICE * (c + 1)]
        m["wsd"] = _pack_wdT(sds, SIT)
        m["xt"] = xt_arr
        in_maps.append(m)

    nc = _build(caps)
    kw = {}
    if _trace:
        kw = dict(trace=True, trace_cores=list(range(NCORES)))
    res = run_bass_kernel_spmd(nc, in_maps, core_ids=list(range(NCORES)), **kw)

    y = np.zeros((T, D), dtype=np.float32)
    for c in range(NCORES):
        y += res.results[c]["ys"].reshape(D, T).T.astype(np.float32)
    for c in range(NCORES):
        for s in range(S):
            e, ptoks, _ = class_pieces[s][c]
            n = len(ptoks)
            if n:
                y[ptoks] += res.results[c][f"yr{s}"].reshape(D, caps[s])[:, :n].T.astype(np.float32)
    if _trace:
        return y, res
    return y


# revision 16
# speedup vs baseline: 1.0027x; 1.0027x over previous
"""DeepseekV2 MoE layer on 8 Trainium2 NeuronCores (Bass/Tile, SPMD).

Strategy (expert-parallel, all-bf16 matmuls, fp32 PSUM accumulate):
 - Host computes the MoE gate routing in numpy (bitwise-matches the jax
   reference: top-k margins are ~1e-4, far above ulp noise).
 - 16 experts -> 8 cores x 3 slots.  Slot capacities (C0>=C1>=C2) are
   chosen by a small DP so that expert token loads (which are heavily
   imbalanced) pack into the 24 slots with minimal total capacity;
   overfull experts are SPLIT across slots on different cores.  SPMD
   cores run an identical instruction stream, so per-core cost is
   C0+C1+C2 regardless of data - minimizing that sum minimizes time.
 - Per slot: GEMM1 (wgu tiles stationary, gathered x^T moving) -> SiLU
   on ScalarE -> *up *cw on VectorE -> GEMM2 *transposed* (w_down^T
   tiles stationary, activations moving) producing y^T [D, cap] so
   capacities need no 128-row padding.  The routed gate weight (incl.
   the 2.5 scale) is applied via a host-replicated [128, cap] row.
 - Shared expert: TP-sharded over its intermediate dim (352/core,
   padded to 3x128), same transposed pipeline, partials summed on host.
 - bf16 operands halve HBM traffic vs fp32 (the baseline bottleneck);
   matmul rate is 1 cycle/row either way.  Measured rel err ~4e-3.
 - Host scatter-adds per-piece outputs and sums shared partials.
"""

import itertools
import numpy as np
import ml_dtypes
from contextlib import ExitStack

import concourse.bacc as bacc
import concourse.tile as tile
import concourse.mybir as mybir
from concourse.bass_utils import run_bass_kernel_spmd

# problem dims (fixed by the graded problem)
T, D, I, E = 1024, 2048, 1408, 16
SI = 2 * I               # shared expert intermediate (2816)
TOP_K, N_GROUP, TOPK_GROUP = 6, 4, 2
ROUTED_SCALE = 2.5
NCORES = 8
KT = D // 128            # 16 contraction tiles (gemm1)
IT = I // 128            # 11 intermediate tiles (routed)
DT = D // 128            # 16 output d-tiles (gemm2)
SSLICE = SI // NCORES    # 352 shared-intermediate rows per core
SIP = 384                # padded to 3x128
SIT = SIP // 128         # 3

f32 = mybir.dt.float32
bf16 = mybir.dt.bfloat16
BF = ml_dtypes.bfloat16
ACT_SILU = mybir.ActivationFunctionType.Silu


# ---------------------------------------------------------------- routing
def _route(x, gate_w, bias):
    """Replicates the jax reference gate in numpy f32 (decision margins are
    >=1e-4 so ulp-level differences cannot flip the top-k).

    Returns topk_idx [T,6] int, weights [T,6] f32 (renormalized, unscaled).
    """
    logits = (x @ gate_w.T).astype(np.float32)
    scores = (1.0 / (1.0 + np.exp(-logits))).astype(np.float32)
    s_choice = scores + bias.astype(np.float32)
    grp = s_choice.reshape(T, N_GROUP, E // N_GROUP)
    group_scores = np.sort(grp, axis=2)[:, :, -2:].sum(2, dtype=np.float32)
    grp_idx = np.argsort(-group_scores, axis=1, kind="stable")[:, :TOPK_GROUP]
    gmask = np.zeros((T, N_GROUP), dtype=bool)
    gmask[np.arange(T)[:, None], grp_idx] = True
    emask = np.repeat(gmask, E // N_GROUP, axis=1)
    masked = np.where(emask, s_choice, -np.inf)
    topk_idx = np.argsort(-masked, axis=1, kind="stable")[:, :TOP_K]
    w = np.take_along_axis(scores, topk_idx, axis=1)
    w = (w / w.sum(axis=1, keepdims=True)).astype(np.float32)
    return topk_idx, w


# --------------------------------------------------------- slot assignment
def _feasible(caps, counts, max_pieces=3):
    """Can each expert be covered by <=max_pieces slots (8 per capacity
    class) with total capacity >= its token count?  Returns per-expert
    class-use tuples or None."""
    nclass = len(caps)
    opts_per_expert = []
    for cnt in counts:
        opts = []
        for a in itertools.product(range(max_pieces + 1), repeat=nclass):
            if sum(a) == 0 or sum(a) > max_pieces:
                continue
            tot = sum(ai * c for ai, c in zip(a, caps))
            if tot >= cnt:
                dom = any(a[j] > 0 and tot - caps[j] >= cnt for j in range(nclass))
                if not dom:
                    opts.append(a)
        if not opts:
            return None
        opts_per_expert.append(opts)
    states = {(0,) * nclass: []}
    for opts in opts_per_expert:
        new = {}
        for st, hist in states.items():
            for a in opts:
                nst = tuple(s + ai for s, ai in zip(st, a))
                if all(n <= NCORES for n in nst) and nst not in new:
                    new[nst] = hist + [a]
        if not new:
            return None
        states = new
    return next(iter(states.values()))


def _solve_slots(counts):
    """Pick 3 slot capacities (multiples of 8, each <=512) minimizing
    C0+C1+C2 s.t. the expert loads pack into 8 slots per class.
    Returns (caps, per-expert class-use tuples)."""
    total = int(counts.sum())
    lo = -(-total // NCORES)
    lo = -(-lo // 8) * 8
    for s in range(lo, 1537, 8):
        for C0 in range(-(-s // 3) // 8 * 8, min(512, s - 16) + 1, 8):
            for C1 in range(8, min(C0, s - C0 - 8) + 1, 8):
                C2 = s - C0 - C1
                if C2 < 8 or C2 > C1:
                    continue
                r = _feasible((C0, C1, C2), counts)
                if r is not None:
                    return [C0, C1, C2], r
    raise RuntimeError("no feasible slot packing")


# ------------------------------------------------------------ host packing
def _pack_wgu(w2, it_cnt):
    """w2: [2*ic, D] rows (gate block then up block, ic=128*it_cnt rows)
    -> [it_cnt, 128, 2, KT, 128] bf16: tile (t, j) is wgu^T laid out so
    [p, c] = w2[j_block + t*128 + c, k*128 + p] (lhsT k-slices)."""
    ic = 128 * it_cnt
    g = w2[:ic].reshape(it_cnt, 128, D)
    u = w2[ic:].reshape(it_cnt, 128, D)
    arr = np.stack([g, u], axis=1)               # (t, j, c, D)
    arr = arr.reshape(it_cnt, 2, 128, KT, 128)   # (t, j, c, k, p)
    return np.ascontiguousarray(arr.transpose(0, 4, 1, 3, 2), dtype=BF)


def _pack_wdT(wd, it_cnt):
    """wd: [D, 128*it_cnt] (w_down, cols may be zero padded)
    -> [DT, 128, it_cnt, 128] bf16: tile (dt, k) is lhsT with
    [i, d] = wd[dt*128 + d, k*128 + i]."""
    a = wd.reshape(DT, 128, it_cnt, 128)         # (dt, d, k, i)
    return np.ascontiguousarray(a.transpose(0, 3, 2, 1), dtype=BF)


def _pack_xT(xs, cap):
    """xs: [n, D] token rows -> [128, KT, cap] bf16 (x^T k-tiles, padded)."""
    out = np.zeros((128, KT, cap), dtype=BF)
    n = xs.shape[0]
    if n:
        out[:, :, :n] = xs.T.reshape(KT, 128, n).transpose(1, 0, 2).astype(BF)
    return out


# ------------------------------------------------------------ device build
def _build(caps):
    nc = bacc.Bacc("TRN2", target_bir_lowering=False, debug=False,
                   num_devices=NCORES)
    S = len(caps)
    xg_d = [nc.dram_tensor(f"xg{s}", [128, KT, caps[s]], bf16, kind="ExternalInput") for s in range(S)]
    cw_d = [nc.dram_tensor(f"cw{s}", [128, caps[s]], f32, kind="ExternalInput") for s in range(S)]
    wgu_d = [nc.dram_tensor(f"wgu{s}", [IT, 128, 2, KT, 128], bf16, kind="ExternalInput") for s in range(S)]
    wd_d = [nc.dram_tensor(f"wd{s}", [DT, 128, IT, 128], bf16, kind="ExternalInput") for s in range(S)]
    yr_d = [nc.dram_tensor(f"yr{s}", [DT, 128, caps[s]], bf16, kind="ExternalOutput") for s in range(S)]
    xt_d = nc.dram_tensor("xt", [128, KT, T], bf16, kind="ExternalInput")
    wsgu_d = nc.dram_tensor("wsgu", [SIT, 128, 2, KT, 128], bf16, kind="ExternalInput")
    wsd_d = nc.dram_tensor("wsd", [DT, 128, SIT, 128], bf16, kind="ExternalInput")
    ys_d = nc.dram_tensor("ys", [DT, 128, T], bf16, kind="ExternalOutput")

    with tile.TileContext(nc) as tc, ExitStack() as ctx:
        sb = ctx.enter_context(tc.tile_pool(name="sb", bufs=1))
        ps = ctx.enter_context(tc.tile_pool(name="ps", bufs=1, space="PSUM"))

        # Warm-up: ~40 dummy matmuls on zeros during the initial DMA wait
        # keep the PE HAM activity window busy, so real work starts at the
        # warm 2.4GHz clock instead of ramping from 1.2GHz.
        warm = sb.tile([128, 256], bf16, tag="warm", bufs=1, name="warm")
        nc.vector.memset(warm[:], 0)
        pwarm = ps.tile([128, 256], f32, tag="psy", bufs=2, name="pwarm")
        for _ in range(40):
            nc.tensor.matmul(pwarm[:], warm[:, :128], warm[:],
                             start=True, stop=True)

        # Token inputs are issued deadline-ordered on the scalar queue:
        # phase 0's x up front, each later phase's x during the previous
        # phase's gemm2 (when the wgu weight stream is idle) to avoid
        # oversubscribing HBM while gemm1 streams weights.
        q = KT // 4
        h = KT // 2
        xgs = [sb.tile([128, KT, caps[s]], bf16, tag="xg", bufs=3, name=f"xga{s}")
               for s in range(S)]
        cws = [sb.tile([128, caps[s]], f32, tag="cw", bufs=3, name=f"cwa{s}")
               for s in range(S)]
        xt = sb.tile([128, KT, T], bf16, tag="xt", bufs=1, name="xt")

        def load_xg(s):
            for i in range(4):
                nc.scalar.dma_start(xgs[s][:, i * q:(i + 1) * q, :],
                                    xg_d[s].ap()[:, i * q:(i + 1) * q, :])
            nc.scalar.dma_start(cws[s][:], cw_d[s].ap()[:])

        def load_xt():
            nc.scalar.dma_start(xt[:, :h, :], xt_d.ap()[:, :h, :])
            nc.scalar.dma_start(xt[:, h:, :], xt_d.ap()[:, h:, :])

        load_xg(0)
        # ALL gemm2 weights hoisted on the dedicated gpsimd queue; bufs=6
        # paces the stream: ~6 tiles prefetch ahead, the rest follow the
        # previous gemm2's progress.  Nothing else shares this queue, so
        # its in-order stalls are harmless.
        wd_tiles = []
        for wdd, itc in [(w, IT) for w in wd_d] + [(wsd_d, SIT)]:
            tiles = []
            for dt in range(DT):
                wd = sb.tile([128, itc, 128], bf16, tag="wd", bufs=6, name="wd")
                nc.gpsimd.dma_start(wd[:], wdd.ap()[dt])
                tiles.append(wd)
            wd_tiles.append(tiles)

        def ffn(xg, cwB, wgu_dram, wds, out_dram, C, it_cnt, chunks,
                prefetch=None, out_dt=f32, first=False):
            # GEMM1 + silu*up*cw -> at (activations^T, [i, tokens], bf16)
            at = sb.tile([128, it_cnt, C], bf16, tag="at", bufs=2, name="at")
            for t in range(it_cnt):
                wgu = sb.tile([128, 2, KT, 128], bf16, tag="wgu", bufs=6, name="wgu")
                if first and t == 0:
                    hh = KT // 2
                    nc.sync.dma_start(wgu[:, 0, :hh, :], wgu_dram.ap()[t][:, 0, :hh, :])
                    nc.sync.dma_start(wgu[:, 0, hh:, :], wgu_dram.ap()[t][:, 0, hh:, :])
                else:
                    nc.sync.dma_start(wgu[:, 0], wgu_dram.ap()[t][:, 0])
                nc.sync.dma_start(wgu[:, 1], wgu_dram.ap()[t][:, 1])
                for off, n in chunks:
                    psg = ps.tile([128, n], f32, tag="psg", bufs=3, name="psg")
                    psu = ps.tile([128, n], f32, tag="psu", bufs=3, name="psu")
                    for j, p in ((0, psg), (1, psu)):
                        for k in range(KT):
                            nc.tensor.matmul(p[:], wgu[:, j, k, :],
                                             xg[:, k, off:off + n],
                                             start=(k == 0), stop=(k == KT - 1))
                    tmp = sb.tile([128, n], f32, tag="tmp", bufs=2, name="tmp")
                    nc.scalar.activation(tmp[:], psg[:], ACT_SILU)
                    if cwB is not None:
                        tmp2 = sb.tile([128, n], f32, tag="tmp2", bufs=2, name="tmp2")
                        nc.vector.tensor_mul(tmp2[:], tmp[:], psu[:])
                        nc.vector.tensor_mul(at[:, t, off:off + n], tmp2[:],
                                             cwB[:, off:off + n])
                    else:
                        nc.vector.tensor_mul(at[:, t, off:off + n], tmp[:], psu[:])
            if prefetch is not None:
                prefetch()
            # GEMM2 (transposed): y^T[d-tile] = sum_k wdT[dt,k].T @ at[k]
            for dt in range(DT):
                wd = wds[dt]
                for off, n in chunks:
                    psy = ps.tile([128, n], f32, tag="psy", bufs=2, name="psy")
                    for k in range(it_cnt):
                        nc.tensor.matmul(psy[:], wd[:, k, :], at[:, k, off:off + n],
                                         start=(k == 0), stop=(k == it_cnt - 1))
                    ysb = sb.tile([128, n], out_dt, tag="ysb", bufs=3, name="ysb")
                    nc.vector.tensor_copy(ysb[:], psy[:])
                    nc.scalar.dma_start(out_dram.ap()[dt][:, off:off + n], ysb[:])

        prefetches = [lambda: load_xg(1), lambda: (load_xg(2), load_xt()),
                      None, None]
        for s in range(S):
            ffn(xgs[s], cws[s], wgu_d[s], wd_tiles[s], yr_d[s], caps[s], IT,
                [(0, caps[s])], prefetch=prefetches[s], out_dt=bf16,
                first=(s == 0))
        ffn(xt, None, wsgu_d, wd_tiles[S], ys_d, T, SIT, [(0, 512), (512, 512)],
            out_dt=bf16)

    nc.compile()
    return nc


# ----------------------------------------------------------------- kernel
def kernel(x, gate_w, bias, w_gate_up, w_down, shared_w_gate_up,
           shared_w_down, _trace=False):
    x = np.ascontiguousarray(x, dtype=np.float32)
    topk_idx, w = _route(x, gate_w, bias)
    cw_full = w.astype(np.float32) * np.float32(ROUTED_SCALE)

    # expert -> token list + weight list
    toks, wts, counts = [], [], np.zeros(E, dtype=np.int64)
    for e in range(E):
        tsel, ksel = np.where(topk_idx == e)
        toks.append(tsel)
        wts.append(cw_full[tsel, ksel])
        counts[e] = len(tsel)

    caps, uses = _solve_slots(counts)
    S = len(caps)
    # build per-class piece lists: (expert, token_idx_array, weight_array)
    class_pieces = [[] for _ in range(S)]
    for e in range(E):
        pos = 0
        # fill this expert's pieces largest-class first
        for s in range(S):
            for _ in range(uses[e][s]):
                n = min(caps[s], counts[e] - pos)
                class_pieces[s].append((e, toks[e][pos:pos + n], wts[e][pos:pos + n]))
                pos += n
    for s in range(S):
        while len(class_pieces[s]) < NCORES:   # dummy empty pieces
            class_pieces[s].append((0, toks[0][:0], wts[0][:0]))

    # pre-pack each expert's weights once (pieces share the arrays)
    wgu_pack = {}
    wd_pack = {}
    for s in range(S):
        for e, _, _ in class_pieces[s]:
            if e not in wgu_pack:
                wgu_pack[e] = _pack_wgu(w_gate_up[e], IT)
                wd_pack[e] = _pack_wdT(np.ascontiguousarray(w_down[e]), IT)
    xt_arr = _pack_xT(x, T)

    in_maps = []
    for c in range(NCORES):
        m = {}
        for s in range(S):
            e, ptoks, pwts = class_pieces[s][c]
            m[f"xg{s}"] = _pack_xT(x[ptoks], caps[s])
            cwb = np.zeros((128, caps[s]), dtype=np.float32)
            cwb[:, :len(pwts)] = pwts[None, :]
            m[f"cw{s}"] = cwb
            m[f"wgu{s}"] = wgu_pack[e]
            m[f"wd{s}"] = wd_pack[e]
        # shared expert slice (rows [352c, 352c+352), zero-padded to 384)
        gsl = np.zeros((2 * SIP, D), dtype=np.float32)
        gsl[:SSLICE] = shared_w_gate_up[SSLICE * c: SSLICE * (c + 1)]
        gsl[SIP:SIP + SSLICE] = shared_w_gate_up[SI + SSLICE * c: SI + SSLICE * (c + 1)]
        m["wsgu"] = _pack_wgu(gsl, SIT)
        sds = np.zeros((D, SIP), dtype=np.float32)
        sds[:, :SSLICE] = shared_w_down[:, SSLICE * c: SSLICE * (c + 1)]
        m["wsd"] = _pack_wdT(sds, SIT)
        m["xt"] = xt_arr
        in_maps.append(m)

    nc = _build(caps)
    kw = {}
    if _trace:
        kw = dict(trace=True, trace_cores=list(range(NCORES)))
    res = run_bass_kernel_spmd(nc, in_maps, core_ids=list(range(NCORES)), **kw)

    y = np.zeros((T, D), dtype=np.float32)
    for c in range(NCORES):
        y += res.results[c]["ys"].reshape(D, T).T.astype(np.float32)
    for c in range(NCORES):
        for s in range(S):
            e, ptoks, _ = class_pieces[s][c]
            n = len(ptoks)
            if n:
                y[ptoks] += res.results[c][f"yr{s}"].reshape(D, caps[s])[:, :n].T.astype(np.float32)
    if _trace:
        return y, res
    return y
